# revision 6
# baseline (speedup 1.0000x reference)
"""Trainium2 Bass kernel for nn_KGRAMS (dual CNN-attention entity nets + LSTM).

Sharding: data-parallel over B=256 user-item pairs across 8 cores (32/core).
Embedding tables + weights replicated. Word-embedding gathers on device via
dma_gather (bf16, transpose mode -> [E, words] layout directly). int16 index
range handled by a two-pass split table with zero rows at each pass base:
out-of-pass indices point at the zero row, passes are merged with one add.
"""
import numpy as np
import ml_dtypes

V, E, C, F, B, R, L = 50000, 128, 100, 3, 256, 10, 80
UID, IID, D, H, LAT = 10000, 10000, 64, 128, 64
NCORES = 8
BC = B // NCORES          # 32 pairs per core
P = 128
NWORDS = BC * R * L       # 25600 review words per entity per core
TWORDS = BC * L           # 2560 target review words per core
GCHUNK = 3200             # dma_gather num_idxs per call (single_packet=False)
SPLIT = 32767             # pass A rows 1..32766; pass B base at row SPLIT

_f32 = np.float32
_bf16 = ml_dtypes.bfloat16
_COMPILED = {}


def _wrap_idx(a):
    n = a.shape[0]
    w = a.reshape(n // 16, 16).T
    return np.ascontiguousarray(np.tile(w, (8, 1)), dtype=np.int16)


def _split_idx(idx):
    """idx [n] in [0,V) -> (idxA, idxB) int16; invalid -> 0 (zero row)."""
    idxA = np.where(idx < SPLIT - 1, idx + 1, 0).astype(np.int16)
    idxB = np.where(idx >= SPLIT - 1, idx - (SPLIT - 1) + 1, 0).astype(np.int16)
    return idxA, idxB


def _pad_table(emb):
    v, e = emb.shape
    tz = np.zeros((v + 2, e), _f32)
    tz[1:SPLIT] = emb[0:SPLIT - 1]
    tz[SPLIT + 1:SPLIT + 1 + (v - (SPLIT - 1))] = emb[SPLIT - 1:]
    return tz.astype(_bf16)


def _build_kernel():
    import concourse.bass as bass
    import concourse.bacc as bacc
    import concourse.mybir as mybir
    import concourse.tile as tile
    from concourse.masks import make_identity

    dt = mybir.dt
    AF = mybir.ActivationFunctionType
    AX = mybir.AxisListType

    nc = bacc.Bacc(None, target_bir_lowering=False)

    VP = V + 2
    t_we = nc.dram_tensor("we", [VP, E], dt.bfloat16, kind="ExternalInput")
    inp = {}

    def din(name, shape, d=dt.float32):
        inp[name] = nc.dram_tensor(name, shape, d, kind="ExternalInput")

    NW16 = NWORDS // 16
    NT16 = TWORDS // 16
    for ent in ("u", "i"):
        din(f"revA_{ent}", [P, NW16], dt.int16)
        din(f"revB_{ent}", [P, NW16], dt.int16)
        din(f"sid_{ent}", [P, 3], dt.int32)      # 320 score ids padded to 384
        din(f"tid_{ent}", [BC, 1], dt.int32)
        din(f"convT_{ent}", [E, F * C])
        din(f"convb_{ent}", [C, 1])
        din(f"WOT_{ent}", [C, H])
        din(f"WUT_{ent}", [D, H])
        din(f"b1_{ent}", [H, 1])
        din(f"hvec_{ent}", [H, 1])
        din(f"linWT_{ent}", [C, LAT])
        din(f"linb_{ent}", [LAT, 1])
        din(f"semb_{ent}", [IID, D])
        din(f"idemb_{ent}", [UID, D])
    din("trevA", [P, NT16], dt.int16)
    din("trevB", [P, NT16], dt.int16)
    din("W1", [P, 1])
    din("bias_pred", [1, BC])
    din("tgt", [1, BC])
    din("c0wT", [P, 2 * H])
    din("h0wT", [P, 2 * H])
    din("c0b", [H, 1])
    din("h0b", [H, 1])
    din("WihT", [E, 4 * H])
    din("WhhT", [H, 4 * H])
    din("bih", [H, 4])

    t_pred = nc.dram_tensor("o_pred", [1, BC], dt.float32, kind="ExternalOutput")
    t_sse = nc.dram_tensor("o_sse", [1, 1], dt.float32, kind="ExternalOutput")
    t_h = nc.dram_tensor("o_h", [BC, H], dt.float32, kind="ExternalOutput")
    t_c = nc.dram_tensor("o_c", [BC, H], dt.float32, kind="ExternalOutput")
    scratch = nc.dram_tensor("scr", [2, BC * R * D], dt.float32, kind="Internal")

    with tile.TileContext(nc) as tc:
        with (
            tc.tile_pool(name="gat", bufs=2) as gat,
            tc.tile_pool(name="idxp", bufs=3) as idxp,
            tc.tile_pool(name="big", bufs=1) as big,
            tc.tile_pool(name="wrk", bufs=2) as wrk,
            tc.tile_pool(name="sml", bufs=2) as sml,
            tc.tile_pool(name="cst", bufs=1) as cst,
            tc.tile_pool(name="pcv", bufs=2, space="PSUM") as pcvp,   # 2x2 banks
            tc.tile_pool(name="pmi", bufs=2, space="PSUM") as pmip,   # 2x1 bank
            tc.tile_pool(name="psm", bufs=2, space="PSUM") as psmp,   # 2x1 bank
        ):
            def ld(name, shape, d=dt.float32):
                t = cst.tile(shape, d, tag=name)
                nc.sync.dma_start(out=t[:], in_=inp[name][:])
                return t

            par = {}
            for ent in ("u", "i"):
                for nm, shp in (
                    (f"convT_{ent}", [E, F * C]), (f"convb_{ent}", [C, 1]),
                    (f"WOT_{ent}", [C, H]), (f"WUT_{ent}", [D, H]),
                    (f"b1_{ent}", [H, 1]), (f"hvec_{ent}", [H, 1]),
                    (f"linWT_{ent}", [C, LAT]), (f"linb_{ent}", [LAT, 1]),
                ):
                    par[nm] = ld(nm, shp)
            for nm, shp in (("W1", [P, 1]), ("bias_pred", [1, BC]), ("tgt", [1, BC]),
                            ("c0wT", [P, 2 * H]), ("h0wT", [P, 2 * H]),
                            ("c0b", [H, 1]), ("h0b", [H, 1]),
                            ("WihT", [E, 4 * H]), ("WhhT", [H, 4 * H]),
                            ("bih", [H, 4])):
                par[nm] = ld(nm, shp)

            convT_bf = {}
            for ent in ("u", "i"):
                cb = cst.tile([E, F * C], dt.bfloat16, tag=f"convbf_{ent}")
                nc.vector.tensor_copy(out=cb[:], in_=par[f"convT_{ent}"][:])
                convT_bf[ent] = cb
            WihT_bf = cst.tile([E, 4 * H], dt.bfloat16, tag="wihbf")
            nc.vector.tensor_copy(out=WihT_bf[:], in_=par["WihT"][:])
            ones_row = cst.tile([1, P], dt.float32, tag="ones")
            nc.gpsimd.memset(ones_row[:], 1.0)
            ident = cst.tile([P, P], dt.float32, tag="ident")
            make_identity(nc, ident[:])

            def gather_words(dst, nmA, nmB, nwords):
                """dst [E, nwords] bf16: pass A straight in, pass B added."""
                start = 0
                while start < nwords:
                    sz = min(GCHUNK, nwords - start)
                    cs = slice(start // 16, (start + sz) // 16)
                    ixA = idxp.tile([P, GCHUNK // 16], dt.int16, tag="ixA")
                    ixB = idxp.tile([P, GCHUNK // 16], dt.int16, tag="ixB")
                    nc.sync.dma_start(out=ixA[:, :sz // 16], in_=inp[nmA][:, cs])
                    nc.sync.dma_start(out=ixB[:, :sz // 16], in_=inp[nmB][:, cs])
                    dslc = dst[:, start:start + sz].rearrange(
                        "e (o n) -> e o n", o=1)
                    nc.gpsimd.dma_gather(
                        out_ap=dslc, in_ap=t_we[:],
                        idxs_ap=ixA[:, :sz // 16], num_idxs=sz, num_idxs_reg=sz,
                        elem_size=E, transpose=True, single_packet=False)
                    gB = gat.tile([P, 1, GCHUNK], dt.bfloat16, tag="gB")
                    nc.gpsimd.dma_gather(
                        out_ap=gB[:, :, :sz], in_ap=t_we[SPLIT:, :],
                        idxs_ap=ixB[:, :sz // 16], num_idxs=sz, num_idxs_reg=sz,
                        elem_size=E, transpose=True, single_packet=False)
                    nc.vector.tensor_add(
                        out=dst[:, start:start + sz],
                        in0=dst[:, start:start + sz], in1=gB[:, 0, :sz])
                    start += sz

            # ================= entity nets =================
            ent_out = {}
            for ei, ent in enumerate(("u", "i")):
                revT = big.tile([E, NWORDS], dt.bfloat16, tag=f"revT_{ent}")
                gather_words(revT, f"revA_{ent}", f"revB_{ent}", NWORDS)

                rf = big.tile([C, BC * R], dt.float32, tag=f"rf_{ent}")
                cw = convT_bf[ent]
                for b in range(BC):
                    pcv = pcvp.tile([P, 800], dt.float32, tag="pcv")
                    base = b * 800
                    for (o0, o1) in ((0, 512), (512, 798)):
                        for f in range(F):
                            nc.tensor.matmul(
                                out=pcv[:C, o0:o1],
                                lhsT=cw[:, f * C:(f + 1) * C],
                                rhs=revT[:, base + o0 + f: base + o1 + f],
                                start=(f == 0), stop=(f == F - 1))
                    mx = sml.tile([C, R], dt.float32, tag="mx")
                    nc.vector.reduce_max(
                        out=mx[:],
                        in_=pcv[:C].rearrange("c (r l) -> c r l", r=R)[:, :, 0:78],
                        axis=AX.X)
                    nc.scalar.activation(
                        out=rf[:, b * R:(b + 1) * R], in_=mx[:],
                        func=AF.Relu, bias=par[f"convb_{ent}"][:])

                # score-emb gather + faithful reshape via DRAM bounce
                sid = sml.tile([P, 3], dt.int32, tag="sid")
                nc.sync.dma_start(out=sid[:], in_=inp[f"sid_{ent}"][:])
                for g in range(3):
                    rows = P if g < 2 else BC * R - 2 * P
                    gse = wrk.tile([P, D], dt.float32, tag="gse")
                    nc.gpsimd.indirect_dma_start(
                        out=gse[:], out_offset=None,
                        in_=inp[f"semb_{ent}"][:],
                        in_offset=bass.IndirectOffsetOnAxis(
                            ap=sid[:, g:g + 1], axis=0))
                    nc.sync.dma_start(
                        out=scratch[ei, g * P * D:(g * P + rows) * D],
                        in_=gse[:rows])
                se = big.tile([D, BC * R], dt.float32, tag=f"se_{ent}")
                nc.sync.dma_start(
                    out=se[:].rearrange("d (b r) -> d b r", r=R),
                    in_=scratch[ei].rearrange("(b d r) -> d b r", b=BC, d=D, r=R))

                pa = pmip.tile([H, BC * R], dt.float32, tag="pmi")
                nc.tensor.matmul(out=pa[:], lhsT=par[f"WOT_{ent}"][:], rhs=rf[:],
                                 start=True, stop=False)
                nc.tensor.matmul(out=pa[:], lhsT=par[f"WUT_{ent}"][:], rhs=se[:],
                                 start=False, stop=True)
                a_sb = wrk.tile([H, BC * R], dt.float32, tag="a_sb")
                nc.scalar.activation(out=a_sb[:], in_=pa[:], func=AF.Relu,
                                     bias=par[f"b1_{ent}"][:])
                patt = psmp.tile([1, BC * R], dt.float32, tag="psm")
                nc.tensor.matmul(out=patt[:], lhsT=par[f"hvec_{ent}"][:],
                                 rhs=a_sb[:], start=True, stop=True)
                ex = sml.tile([1, BC * R], dt.float32, tag="ex")
                nc.scalar.activation(out=ex[:], in_=patt[:], func=AF.Exp)
                ssum = sml.tile([1, BC], dt.float32, tag="ssum")
                nc.vector.reduce_sum(
                    out=ssum[:], in_=ex[:].rearrange("o (b r) -> o b r", r=R),
                    axis=AX.X)
                rcp = sml.tile([1, BC], dt.float32, tag="rcp")
                nc.vector.reciprocal(out=rcp[:], in_=ssum[:])
                attn = sml.tile([1, BC * R], dt.float32, tag="attn")
                nc.vector.tensor_mul(
                    out=attn[:].rearrange("o (b r) -> o b r", r=R),
                    in0=ex[:].rearrange("o (b r) -> o b r", r=R),
                    in1=rcp[:].rearrange("o (b x) -> o b x", x=1)
                        .to_broadcast([1, BC, R]))
                pattb = pmip.tile([P, BC * R], dt.float32, tag="pmi")
                nc.tensor.matmul(out=pattb[:], lhsT=ones_row[:], rhs=attn[:],
                                 start=True, stop=True)
                wrf = wrk.tile([C, BC * R], dt.float32, tag="wrf")
                nc.vector.tensor_mul(out=wrf[:], in0=rf[:], in1=pattb[:C])
                imp = sml.tile([C, BC], dt.float32, tag="imp")
                nc.vector.reduce_sum(
                    out=imp[:], in_=wrf[:].rearrange("c (b r) -> c b r", r=R),
                    axis=AX.X)

                eout = big.tile([P, BC], dt.float32, tag=f"ent_{ent}")
                pef = psmp.tile([LAT, BC], dt.float32, tag="psm")
                nc.tensor.matmul(out=pef[:], lhsT=par[f"linWT_{ent}"][:],
                                 rhs=imp[:], start=True, stop=True)
                nc.scalar.activation(out=eout[D:, :], in_=pef[:],
                                     func=AF.Identity, bias=par[f"linb_{ent}"][:])
                tid = sml.tile([BC, 1], dt.int32, tag="tid")
                nc.sync.dma_start(out=tid[:], in_=inp[f"tid_{ent}"][:])
                gid = sml.tile([BC, D], dt.float32, tag="gid")
                nc.gpsimd.indirect_dma_start(
                    out=gid[:], out_offset=None, in_=inp[f"idemb_{ent}"][:],
                    in_offset=bass.IndirectOffsetOnAxis(ap=tid[:, :1], axis=0))
                ptid = psmp.tile([D, BC], dt.float32, tag="psm")
                nc.tensor.transpose(out=ptid[:], in_=gid[:],
                                    identity=ident[:BC, :BC])
                nc.vector.tensor_copy(out=eout[:D, :], in_=ptid[:])
                ent_out[ent] = eout

            uf, itf = ent_out["u"], ent_out["i"]

            # ================= rating head =================
            uif = wrk.tile([P, BC], dt.float32, tag="uif")
            nc.vector.tensor_mul(out=uif[:], in0=uf[:], in1=itf[:])
            ppred = psmp.tile([1, BC], dt.float32, tag="psm")
            nc.tensor.matmul(out=ppred[:], lhsT=par["W1"][:], rhs=uif[:],
                             start=True, stop=True)
            pred = sml.tile([1, BC], dt.float32, tag="pred")
            nc.vector.tensor_add(out=pred[:], in0=ppred[:], in1=par["bias_pred"][:])
            nc.sync.dma_start(out=t_pred[:], in_=pred[:])
            diff = sml.tile([1, BC], dt.float32, tag="diff")
            nc.vector.tensor_tensor(out=diff[:], in0=pred[:], in1=par["tgt"][:],
                                    op=mybir.AluOpType.subtract)
            sq = sml.tile([1, BC], dt.float32, tag="sq")
            nc.vector.tensor_mul(out=sq[:], in0=diff[:], in1=diff[:])
            sse = sml.tile([1, 1], dt.float32, tag="sse")
            nc.vector.reduce_sum(out=sse[:], in_=sq[:], axis=AX.X)
            nc.sync.dma_start(out=t_sse[:], in_=sse[:])

            # ================= LSTM =================
            hC = wrk.tile([H, BC], dt.float32, tag="hC")
            cC = wrk.tile([H, BC], dt.float32, tag="cC")
            for dst, wnm, bnm in ((cC, "c0wT", "c0b"), (hC, "h0wT", "h0b")):
                pz = psmp.tile([H, BC], dt.float32, tag="psm")
                nc.tensor.matmul(out=pz[:], lhsT=par[wnm][:, :H], rhs=uf[:],
                                 start=True, stop=False)
                nc.tensor.matmul(out=pz[:], lhsT=par[wnm][:, H:], rhs=itf[:],
                                 start=False, stop=True)
                nc.scalar.activation(out=dst[:], in_=pz[:], func=AF.Tanh,
                                     bias=par[bnm][:])

            xT = big.tile([E, TWORDS], dt.bfloat16, tag="xT")
            gather_words(xT, "trevA", "trevB", TWORDS)
            # xproj [H, 4, BC, L]: gate-major, then (b, t) matching xT cols
            xproj = big.tile([H, 4, BC, L], dt.float32, tag="xproj")
            for g in range(4):
                dstg = xproj[:, g].rearrange("h b t -> h (b t)")
                for c0 in range(0, TWORDS, 512):
                    cw_ = min(512, TWORDS - c0)
                    pxp = pmip.tile([H, 512], dt.float32, tag="pmi")
                    nc.tensor.matmul(
                        out=pxp[:, :cw_], lhsT=WihT_bf[:, g * H:(g + 1) * H],
                        rhs=xT[:, c0:c0 + cw_], start=True, stop=True)
                    nc.scalar.activation(
                        out=dstg[:, c0:c0 + cw_], in_=pxp[:, :cw_],
                        func=AF.Identity, bias=par["bih"][:, g:g + 1])

            WhhT = par["WhhT"]
            for t in range(L):
                pg = pmip.tile([H, 4 * BC], dt.float32, tag="pmi")
                for g in range(4):
                    nc.tensor.matmul(
                        out=pg[:, g * BC:(g + 1) * BC],
                        lhsT=WhhT[:, g * H:(g + 1) * H],
                        rhs=hC[:], start=True, stop=True)
                gsum = wrk.tile([H, 4 * BC], dt.float32, tag="gsum")
                nc.vector.tensor_add(
                    out=gsum[:], in0=pg[:],
                    in1=xproj[:, :, :, t].rearrange("h g b -> h (g b)"))
                act = wrk.tile([H, 4 * BC], dt.float32, tag="act")
                nc.scalar.activation(out=act[:, :3 * BC], in_=gsum[:, :3 * BC],
                                     func=AF.Sigmoid)
                nc.scalar.activation(out=act[:, 3 * BC:], in_=gsum[:, 3 * BC:],
                                     func=AF.Tanh)
                fc = sml.tile([H, BC], dt.float32, tag="fc")
                nc.vector.tensor_mul(out=fc[:], in0=act[:, BC:2 * BC], in1=cC[:])
                ig = sml.tile([H, BC], dt.float32, tag="ig")
                nc.vector.tensor_mul(out=ig[:], in0=act[:, :BC],
                                     in1=act[:, 3 * BC:])
                nc.vector.tensor_add(out=cC[:], in0=fc[:], in1=ig[:])
                tch = sml.tile([H, BC], dt.float32, tag="tch")
                nc.scalar.activation(out=tch[:], in_=cC[:], func=AF.Tanh)
                nc.vector.tensor_mul(out=hC[:], in0=act[:, 2 * BC:3 * BC],
                                     in1=tch[:])

            for src, dstt in ((hC, t_h), (cC, t_c)):
                pt = psmp.tile([BC, H], dt.float32, tag="psm")
                nc.tensor.transpose(out=pt[:], in_=src[:], identity=ident[:])
                ot = sml.tile([BC, H], dt.float32, tag="ot")
                nc.vector.tensor_copy(out=ot[:], in_=pt[:])
                nc.sync.dma_start(out=dstt[:], in_=ot[:])

    nc.finalize()
    return nc


def _prep_inputs(params, user_ids, user_reviews, item_ids_of_reviews, item_ids,
                 item_reviews, user_ids_of_reviews, target_ratings,
                 target_reviews):
    we_pad = _pad_table(np.asarray(params["word_emb"], _f32))

    def prep_ent(p):
        convT = np.ascontiguousarray(
            np.asarray(p["conv_w"], _f32)[:, 0].transpose(2, 1, 0)
            .reshape(E, F * C))
        return {
            "convT": convT,
            "convb": np.asarray(p["conv_b"], _f32).reshape(C, 1),
            "WOT": np.ascontiguousarray(np.asarray(p["W_O"], _f32).T),
            "WUT": np.ascontiguousarray(np.asarray(p["W_u"], _f32).T),
            "b1": np.asarray(p["b1"], _f32).reshape(H, 1),
            "hvec": np.asarray(p["h"], _f32).reshape(H, 1),
            "linWT": np.ascontiguousarray(np.asarray(p["lin_w"], _f32).T),
            "linb": np.asarray(p["lin_b"], _f32).reshape(LAT, 1),
            "semb": np.asarray(p["score_emb"], _f32),
            "idemb": np.asarray(p["id_emb"], _f32),
        }

    eu = prep_ent(params["user_net"])
    ei_ = prep_ent(params["item_net"])
    W1 = np.asarray(params["W_1"], _f32).reshape(P, 1)
    bias_all = (np.asarray(params["b_u"], _f32) + np.asarray(params["b_i"], _f32)
                + np.asarray(params["mu"], _f32))
    _c0T = np.asarray(params["c0_w"], _f32).T
    _h0T = np.asarray(params["h0_w"], _f32).T
    c0wT = np.ascontiguousarray(np.concatenate([_c0T[:P], _c0T[P:]], axis=1))
    h0wT = np.ascontiguousarray(np.concatenate([_h0T[:P], _h0T[P:]], axis=1))
    c0b = np.asarray(params["c0_b"], _f32).reshape(H, 1)
    h0b = np.asarray(params["h0_b"], _f32).reshape(H, 1)
    Wih = np.asarray(params["W_ih"], _f32).reshape(4, H, E)
    Whh = np.asarray(params["W_hh"], _f32).reshape(4, H, H)
    bihs = (np.asarray(params["b_ih"], _f32)
            + np.asarray(params["b_hh"], _f32)).reshape(4, H)
    perm = [0, 1, 3, 2]  # torch i,f,g,o -> kernel i,f,o,g
    WihT = np.ascontiguousarray(Wih[perm].reshape(4 * H, E).T)
    WhhT = np.ascontiguousarray(Whh[perm].reshape(4 * H, H).T)
    bih = np.ascontiguousarray(bihs[perm].T)  # [H, 4]

    uids = np.asarray(user_ids).astype(np.int64)
    iids = np.asarray(item_ids).astype(np.int64)
    urev = np.asarray(user_reviews).astype(np.int64)
    irev = np.asarray(item_reviews).astype(np.int64)
    usid = np.asarray(item_ids_of_reviews).astype(np.int64)
    isid = np.asarray(user_ids_of_reviews).astype(np.int64)
    trat = np.asarray(target_ratings, _f32)
    trev = np.asarray(target_reviews).astype(np.int64)

    def prep_sid(a):
        flat = a.reshape(-1).astype(np.int32)          # [320]
        pad = np.zeros(3 * P, np.int32)
        pad[:flat.shape[0]] = flat
        return np.ascontiguousarray(pad.reshape(3, P).T)

    maps = []
    for c in range(NCORES):
        s = slice(c * BC, (c + 1) * BC)
        uA, uB = _split_idx(urev[s].reshape(-1))
        iA, iB = _split_idx(irev[s].reshape(-1))
        tA, tB = _split_idx(trev[s].reshape(-1))
        m = {
            "we": we_pad,
            "revA_u": _wrap_idx(uA), "revB_u": _wrap_idx(uB),
            "revA_i": _wrap_idx(iA), "revB_i": _wrap_idx(iB),
            "trevA": _wrap_idx(tA), "trevB": _wrap_idx(tB),
            "sid_u": prep_sid(usid[s]), "sid_i": prep_sid(isid[s]),
            "tid_u": uids[s].reshape(-1, 1).astype(np.int32),
            "tid_i": iids[s].reshape(-1, 1).astype(np.int32),
            "W1": W1, "bias_pred": bias_all[s].reshape(1, BC),
            "tgt": trat[s].reshape(1, BC),
            "c0wT": c0wT, "h0wT": h0wT, "c0b": c0b, "h0b": h0b,
            "WihT": WihT, "WhhT": WhhT, "bih": bih,
        }
        for ent, ep in (("u", eu), ("i", ei_)):
            for k, v in ep.items():
                m[f"{k}_{ent}"] = v
        maps.append(m)
    return maps


def kernel(params, user_ids, user_reviews, item_ids_of_reviews, item_ids,
           item_reviews, user_ids_of_reviews, target_ratings, target_reviews):
    from concourse.bass_utils import run_bass_kernel_spmd

    if "nc" not in _COMPILED:
        _COMPILED["nc"] = _build_kernel()
    nc = _COMPILED["nc"]
    maps = _prep_inputs(params, user_ids, user_reviews, item_ids_of_reviews,
                        item_ids, item_reviews, user_ids_of_reviews,
                        target_ratings, target_reviews)
    res = run_bass_kernel_spmd(nc, maps, core_ids=list(range(NCORES)))
    preds = np.concatenate([r["o_pred"].reshape(-1) for r in res.results])
    loss = np.array(
        [sum(float(r["o_sse"].reshape(-1)[0]) for r in res.results) / B], _f32)
    hN = np.concatenate([r["o_h"].reshape(-1) for r in res.results])
    cN = np.concatenate([r["o_c"].reshape(-1) for r in res.results])
    return np.concatenate([preds, loss, hN, cN]).astype(_f32)
